# revision 13
# baseline (speedup 1.0000x reference)
"""Multi-head self-attention (no causal mask) on 8 Trainium2 NeuronCores.

Problem: B=2, S=2048, D=768, H=12 heads (head_dim 64), fp32 in/out.
Sharding: batch x head-group. Core c handles batch c//4 and heads
3*(c%4) .. 3*(c%4)+2 (Megatron column-parallel QKV, row-parallel Wo).
Each core computes a partial [2048, 768] output; the host sums the 4
partials per batch and adds bo.

All on-chip compute is bf16 (matmul accumulation fp32 in PSUM), which
keeps end-to-end absmax-rel error ~1e-3 against the fp32 reference.
The host pre-transposes x to x^T and pre-casts all weights to bf16, so
the device does no transposes at all.

Per-core steady state is Activation-engine bound: softmax needs
exp() of 3 heads x 2048^2 = 12.6M scores, and ScalarE runs 1 elem/
cycle/lane at 1.2 GHz regardless of dtype (~110us in [128,1024]
chunks).  Everything else (PE ~100us, DVE ~60us, DMA ~35us) is
overlapped under it:
  - JIT projections: only Q(qh0)+K(k0..7) run before the first score
    matmul, so the first exp issues ~9us in.
  - scores per k-tile: A on PE rows 0-63, B on rows 64-127, C kept at
    rows 64-127 (its K/Q live on partitions 64-127) so consecutive
    matmuls pair on disjoint row groups.
  - attn@V accumulates 4 k-tiles in PSUM ([65,512] chunks, V carrying
    a ones-column for the softmax denominators), then one DVE add into
    an SBUF accumulator.
  - Wo for query-half 0 is woven into query-half 1's k-loop.
"""

import numpy as np

_CACHE = {}

S = 2048
D = 768
HLOC = 3          # heads per core
NKT = 6           # 768 / 128 d-tiles
QH = 1024         # query half width


def _emit(nc, tc, ctx, dram, loop_n=None, phase=4):
    import concourse.mybir as mybir

    f32 = mybir.dt.float32
    bf16 = mybir.dt.bfloat16
    add = mybir.AluOpType.add
    mult = mybir.AluOpType.mult
    Exp = mybir.ActivationFunctionType.Exp

    consts = ctx.enter_context(tc.tile_pool(name="consts", bufs=1))
    ppool = ctx.enter_context(tc.tile_pool(name="ppool", bufs=24))
    pspool = ctx.enter_context(tc.tile_pool(name="pspool", bufs=2, space="PSUM"))
    popool = ctx.enter_context(tc.tile_pool(name="popool", bufs=2, space="PSUM"))
    pjpool = ctx.enter_context(tc.tile_pool(name="pjpool", bufs=2, space="PSUM"))
    opool = ctx.enter_context(tc.tile_pool(name="opool", bufs=2))
    bpool = ctx.enter_context(tc.tile_pool(name="bpool", bufs=2))
    rpool = ctx.enter_context(tc.tile_pool(name="rpool", bufs=2))

    # ---- persistent SBUF tensors ----
    xt = consts.tile([128, NKT, S], bf16)        # x^T (DMA'd pre-transposed)
    qt = consts.tile([128, S], bf16)             # Q^T heads A,B (scaled 1/8)
    kt = consts.tile([128, S], bf16)             # K^T heads A,B
    qtc = consts.tile([128, S], bf16)            # Q^T head C (rows 64:128)
    ktc = consts.tile([128, S], bf16)            # K^T head C (rows 64:128)
    v_sb = consts.tile([128, 16, HLOC, 72], bf16)  # V natural + ones col at 64
    acc = consts.tile([65, HLOC, S], f32)        # attn@V accumulator + denom
    attnT = consts.tile([128, S], bf16)          # normalized attn out^T A,B
    attnTc = consts.tile([64, S], bf16)          # head C

    w_qsb = consts.tile([128, NKT, 128], bf16)
    w_ksb = consts.tile([128, NKT, 128], bf16)
    w_qkc = consts.tile([128, NKT, 128], bf16)   # [Wq_C | Wk_C]
    w_vsb = consts.tile([128, NKT, 192], bf16)
    w_oab = consts.tile([128, D], bf16)
    w_oc = consts.tile([64, D], bf16)
    bq1 = consts.tile([128, 1], f32)
    bq2 = consts.tile([64, 1], f32)
    bk1 = consts.tile([128, 1], f32)
    bkc = consts.tile([128, 1], f32)             # rows 64:128 hold bk_C
    bv_bc = consts.tile([128, HLOC * 64], f32)
    dmy = consts.tile([1, 8], f32)
    dmy2 = consts.tile([1, 8], f32)

    # ---- prologue: warm the exp table while weights stream in ----
    nc.vector.memset(dmy, 0.0)
    nc.scalar.activation(out=dmy2, in_=dmy, func=Exp)

    # weight/bias DMAs ride the Activation queue (idle until first exp)
    nc.scalar.dma_start(out=w_qsb, in_=dram["w_qsb"])
    nc.scalar.dma_start(out=w_ksb, in_=dram["w_ksb"])
    nc.scalar.dma_start(out=w_qkc, in_=dram["w_qkc"])
    nc.scalar.dma_start(out=w_vsb, in_=dram["w_vsb"])
    nc.scalar.dma_start(out=w_oab, in_=dram["wo_ab"])
    nc.scalar.dma_start(out=w_oc, in_=dram["wo_c"])
    nc.scalar.dma_start(out=bq1, in_=dram["bq1"])
    nc.scalar.dma_start(out=bq2, in_=dram["bq2"])
    nc.scalar.dma_start(out=bk1, in_=dram["bk1"])
    nc.scalar.dma_start(out=bkc[64:128, :], in_=dram["bkc"])
    nc.scalar.dma_start(out=bv_bc, in_=dram["bv_bc"])

    ones_bf = consts.tile([128, 16 * HLOC], bf16)
    nc.vector.memset(ones_bf, 1.0)
    nc.vector.tensor_copy(
        out=v_sb[:, :, :, 64:65],
        in_=ones_bf.rearrange("p (a b c) -> p a b c", b=HLOC, c=1))

    xd = dram["xt"]

    def body():
        # ---- input DMAs (sync queue) ----
        for h in range(2):
            for c in range(2):
                cs = slice(h * 1024 + c * 512, h * 1024 + (c + 1) * 512)
                for dt in range(NKT):
                    nc.sync.dma_start(out=xt[:, dt, cs], in_=xd[:, dt, cs])

        # ---- JIT projection helpers ----
        nm = iter(range(10000))

        def proj_qk(wsb, cols, dst, bias, scale=None):
            pp = pjpool.tile([128, 512], f32, name=f"pp_{next(nm)}", tag="w")
            for dt in range(NKT):
                nc.tensor.matmul(pp, lhsT=wsb[:, dt, :], rhs=xt[:, dt, cols],
                                 start=(dt == 0), stop=(dt == NKT - 1))
            if scale is None:
                nc.vector.tensor_scalar_add(dst, pp, bias)
            else:
                nc.vector.tensor_scalar(dst, pp, bias, scale, add, mult)

        def proj_c(c2):
            # merged [Q_C | K_C] for 512 source positions
            cols = slice(c2 * 512, (c2 + 1) * 512)
            pp = pjpool.tile([128, 512], f32, name=f"ppc_{c2}", tag="w")
            for dt in range(NKT):
                nc.tensor.matmul(pp, lhsT=w_qkc[:, dt, :], rhs=xt[:, dt, cols],
                                 start=(dt == 0), stop=(dt == NKT - 1))
            nc.vector.tensor_scalar(qtc[0:64, cols], pp[0:64, :], bq2, 0.125,
                                    add, mult)
            nc.vector.tensor_scalar_add(ktc[64:128, cols], pp[64:128, :],
                                        bkc[64:128, :])
            # move Q_C up to rows 64:128 to pair C's score matmuls there
            nc.vector.tensor_copy(out=qtc[64:128, cols], in_=qtc[0:64, cols])

        def proj_v(sti):
            cols = slice(sti * 128, (sti + 1) * 128)
            pv = pjpool.tile([128, 192], f32, name=f"pv_{sti}", tag="w")
            for dt in range(NKT):
                nc.tensor.matmul(pv, lhsT=xt[:, dt, cols], rhs=w_vsb[:, dt, :],
                                 start=(dt == 0), stop=(dt == NKT - 1))
            nc.vector.tensor_tensor(
                out=v_sb[:, sti, :, 0:64],
                in0=pv.rearrange("p (h d) -> p h d", h=HLOC),
                in1=bv_bc.rearrange("p (h d) -> p h d", h=HLOC),
                op=add)

        # ---- attention pieces ----
        p_tiles = {}

        def scores_exp(kti, qh):
            if phase < 2:
                return
            qs = slice(qh * QH, (qh + 1) * QH)
            ks = slice(kti * 128, (kti + 1) * 128)
            for h, (lh, rh, base) in enumerate(
                    ((kt, qt, 0), (kt, qt, 64), (ktc, qtc, 64))):
                ps = pspool.tile([128, QH], f32, name=f"ps{h}_{kti}_{qh}",
                                 tag="ps")
                for c in range(2):
                    nc.tensor.matmul(
                        ps[:, c * 512:(c + 1) * 512],
                        lhsT=lh[base:base + 64, ks],
                        rhs=rh[base:base + 64,
                               qh * QH + c * 512: qh * QH + (c + 1) * 512],
                        start=True, stop=True)
                p_t = ppool.tile([128, QH], bf16, name=f"p{h}_{kti}_{qh}",
                                 tag="p")
                nc.scalar.activation(out=p_t, in_=ps, func=Exp)
                p_tiles[(h, kti)] = p_t

        def attn_span(kg, qh, c, h):
            """One attn@V accumulation: head h, 512-query chunk c, k-group kg."""
            if phase < 3:
                return
            po = popool.tile([65, 512], f32, name=f"po_{h}_{c}_{kg}_{qh}",
                            tag="po")
            for i, kti in enumerate(range(4 * kg, 4 * kg + 4)):
                nc.tensor.matmul(
                    po, lhsT=v_sb[:, kti, h, 0:65],
                    rhs=p_tiles[(h, kti)][:, c * 512:(c + 1) * 512],
                    start=(i == 0), stop=(i == 3))
            dst = acc[:, h, qh * QH + c * 512: qh * QH + (c + 1) * 512]
            if kg == 0:
                nc.vector.tensor_copy(out=dst, in_=po)
            else:
                nc.vector.tensor_tensor(out=dst, in0=dst, in1=po, op=add)

        def norm_chunk(qh, c):
            if phase < 3:
                return
            qs = slice(qh * QH + c * 512, qh * QH + (c + 1) * 512)
            for h in range(HLOC):
                r_t = rpool.tile([1, 512], f32, name=f"r_{h}_{qh}_{c}", tag="r")
                nc.vector.reciprocal(out=r_t, in_=acc[64:65, h, qs])
                b_t = bpool.tile([64, 512], f32, name=f"b_{h}_{qh}_{c}", tag="b")
                nc.gpsimd.partition_broadcast(b_t, r_t)
                if h == 0:
                    dst = attnT[0:64, qs]
                elif h == 1:
                    dst = attnT[64:128, qs]
                else:
                    dst = attnTc[0:64, qs]
                nc.vector.tensor_tensor(out=dst, in0=acc[0:64, h, qs], in1=b_t,
                                        op=mult)

        def emit_wo(stis):
            if phase < 4:
                return
            for sti in stis:
                ssl = slice(sti * 128, (sti + 1) * 128)
                o_t = opool.tile([128, D], f32, name=f"o_{sti}", tag="o")
                for e in range(2):
                    esl = slice(e * 384, (e + 1) * 384)
                    pw = pjpool.tile([128, 384], f32, name=f"pwo_{sti}_{e}",
                                    tag="w")
                    nc.tensor.matmul(pw, lhsT=attnT[:, ssl], rhs=w_oab[:, esl],
                                     start=True, stop=False)
                    nc.tensor.matmul(pw, lhsT=attnTc[0:64, ssl],
                                     rhs=w_oc[0:64, esl],
                                     start=False, stop=True)
                    nc.vector.tensor_copy(out=o_t[:, esl], in_=pw)
                nc.sync.dma_start(out=dram["out"][ssl, :], in_=o_t)

        # ---- emission order = pipeline order ----
        # minimal prologue: first score matmul (k0, qh0) needs Q(qh0),
        # Q_C(qh0), and K cols 0:512 only
        proj_qk(w_qsb, slice(0, 512), qt[:, 0:512], bq1, 0.125)
        proj_qk(w_qsb, slice(512, 1024), qt[:, 512:1024], bq1, 0.125)
        proj_c(0)
        proj_c(1)
        proj_qk(w_ksb, slice(0, 512), kt[:, 0:512], bk1)

        # Flat k-tile loop over both query halves.  Window w covers the 4
        # k-tiles of group (w % 4) for query half (w // 4); between each
        # k-tile's scores+exp we drain a few deferred items — the previous
        # k-group's attn@V spans and JIT projections the NEXT window needs —
        # so the Activation queue never sits behind a long PE backlog.
        def spans(kg, qh):
            out = []
            for c in range(2):
                for h in range(HLOC):
                    out.append(lambda kg=kg, qh=qh, c=c, h=h:
                               attn_span(kg, qh, c, h))
            return out

        jits = {
            0: [lambda: proj_qk(w_ksb, slice(512, 1024), kt[:, 512:1024], bk1)]
               + [lambda s=s: proj_v(s) for s in range(0, 4)],
            1: [lambda: proj_qk(w_ksb, slice(1024, 1536), kt[:, 1024:1536],
                                bk1),
                lambda: proj_c(2)]
               + [lambda s=s: proj_v(s) for s in range(4, 8)],
            2: [lambda: proj_qk(w_ksb, slice(1536, 2048), kt[:, 1536:2048],
                                bk1),
                lambda: proj_c(3)]
               + [lambda s=s: proj_v(s) for s in range(8, 12)],
            3: [lambda: proj_qk(w_qsb, slice(1024, 1536), qt[:, 1024:1536],
                                bq1, 0.125),
                lambda: proj_qk(w_qsb, slice(1536, 2048), qt[:, 1536:2048],
                                bq1, 0.125)]
               + [lambda s=s: proj_v(s) for s in range(12, 16)],
            4: [lambda: norm_chunk(0, 0), lambda: norm_chunk(0, 1)],
            5: [lambda s=s: emit_wo([s]) for s in range(0, 4)],
            6: [lambda s=s: emit_wo([s]) for s in range(4, 8)],
            7: [],
        }

        for w in range(8):
            qh, kg = divmod(w, 4)
            # deferred: previous window's attn@V spans first (their p tiles
            # must be freed before this window's exps recycle the slots),
            # then this window's JIT projections.
            if w == 0:
                items = list(jits[0])
            elif w == 4:
                # qh boundary: spans of (kg3, qh0), then norm(0) chunks
                items = spans(3, 0) + jits[4]
            else:
                items = spans((w - 1) % 4, (w - 1) // 4) + jits[w]
            for i, kti in enumerate(range(4 * kg, 4 * kg + 4)):
                scores_exp(kti, qh)
                for fn in items[i::4]:
                    fn()

        # tail: last k-group of qh1, chunk-pipelined into norm + Wo
        for h in range(HLOC):
            attn_span(3, 1, 0, h)
        norm_chunk(1, 0)
        for h in range(HLOC):
            attn_span(3, 1, 1, h)
        emit_wo(range(8, 12))
        norm_chunk(1, 1)
        emit_wo(range(12, 16))

    if loop_n is None:
        body()
    else:
        with tc.For_i(0, loop_n, 1):
            body()


def _build(loop_n=None, phase=4):
    from contextlib import ExitStack

    import concourse.bacc as bacc
    import concourse.mybir as mybir
    import concourse.tile as tile

    f32 = mybir.dt.float32
    bf16 = mybir.dt.bfloat16
    nc = bacc.Bacc("TRN2", target_bir_lowering=False, debug=False, num_devices=8)
    dram = {
        "xt": nc.dram_tensor("xt", [128, NKT, S], bf16, kind="ExternalInput").ap(),
        "w_qsb": nc.dram_tensor("w_qsb", [128, NKT, 128], bf16,
                                kind="ExternalInput").ap(),
        "w_ksb": nc.dram_tensor("w_ksb", [128, NKT, 128], bf16,
                                kind="ExternalInput").ap(),
        "w_qkc": nc.dram_tensor("w_qkc", [128, NKT, 128], bf16,
                                kind="ExternalInput").ap(),
        "w_vsb": nc.dram_tensor("w_vsb", [128, NKT, 192], bf16,
                                kind="ExternalInput").ap(),
        "wo_ab": nc.dram_tensor("wo_ab", [128, D], bf16,
                                kind="ExternalInput").ap(),
        "wo_c": nc.dram_tensor("wo_c", [64, D], bf16, kind="ExternalInput").ap(),
        "bq1": nc.dram_tensor("bq1", [128, 1], f32, kind="ExternalInput").ap(),
        "bq2": nc.dram_tensor("bq2", [64, 1], f32, kind="ExternalInput").ap(),
        "bk1": nc.dram_tensor("bk1", [128, 1], f32, kind="ExternalInput").ap(),
        "bkc": nc.dram_tensor("bkc", [64, 1], f32, kind="ExternalInput").ap(),
        "bv_bc": nc.dram_tensor("bv_bc", [128, 192], f32,
                                kind="ExternalInput").ap(),
        "out": nc.dram_tensor("out", [S, D], f32, kind="ExternalOutput").ap(),
    }
    with tile.TileContext(nc) as tc:
        with ExitStack() as ctx:
            _emit(nc, tc, ctx, dram, loop_n=loop_n, phase=phase)
    nc.compile()
    return nc


def _get_nc():
    if "nc" not in _CACHE:
        _CACHE["nc"] = _build()
    return _CACHE["nc"]


def _shard(inputs):
    import ml_dtypes

    bf = ml_dtypes.bfloat16
    x = np.asarray(inputs["x"], np.float32)
    Wq = np.asarray(inputs["Wq"], np.float32)
    Wk = np.asarray(inputs["Wk"], np.float32)
    Wv = np.asarray(inputs["Wv"], np.float32)
    Wo = np.asarray(inputs["Wo"], np.float32)
    bq = np.asarray(inputs["bq"], np.float32)
    bk = np.asarray(inputs["bk"], np.float32)
    bv = np.asarray(inputs["bv"], np.float32)

    def wtiles(w):  # [768, C] -> [128, 6, C]
        return np.ascontiguousarray(
            w.reshape(NKT, 128, -1).transpose(1, 0, 2)).astype(bf)

    xts = []
    for b in range(2):
        xts.append(np.ascontiguousarray(
            x[b].T.reshape(NKT, 128, S).transpose(1, 0, 2)).astype(bf))

    in_maps = []
    for c in range(8):
        b, g = divmod(c, 4)
        o = 192 * g
        in_maps.append({
            "xt": xts[b],
            "w_qsb": wtiles(Wq[:, o:o + 128]),
            "w_ksb": wtiles(Wk[:, o:o + 128]),
            "w_qkc": wtiles(np.concatenate(
                [Wq[:, o + 128:o + 192], Wk[:, o + 128:o + 192]], axis=1)),
            "w_vsb": wtiles(Wv[:, o:o + 192]),
            "wo_ab": np.ascontiguousarray(Wo[o:o + 128, :]).astype(bf),
            "wo_c": np.ascontiguousarray(Wo[o + 128:o + 192, :]).astype(bf),
            "bq1": np.ascontiguousarray(bq[o:o + 128, None]),
            "bq2": np.ascontiguousarray(bq[o + 128:o + 192, None]),
            "bk1": np.ascontiguousarray(bk[o:o + 128, None]),
            "bkc": np.ascontiguousarray(bk[o + 128:o + 192, None]),
            "bv_bc": np.ascontiguousarray(
                np.broadcast_to(bv[o:o + 192], (128, 192))),
        })
    return in_maps


def kernel(x, Wq, bq, Wk, bk, Wv, bv, Wo, bo):
    from concourse.bass_utils import run_bass_kernel_spmd

    nc = _get_nc()
    in_maps = _shard(dict(x=x, Wq=Wq, Wk=Wk, Wv=Wv, Wo=Wo,
                          bq=bq, bk=bk, bv=bv))
    res = run_bass_kernel_spmd(nc, in_maps, core_ids=list(range(8)))
    out = np.zeros((2, S, D), np.float32)
    for c in range(8):
        out[c // 4] += res.results[c]["out"]
    out += np.asarray(bo, np.float32)
    return out


# revision 14
# speedup vs baseline: 1.2087x; 1.2087x over previous
"""Multi-head self-attention (no causal mask) on 8 Trainium2 NeuronCores.

Problem: B=2, S=2048, D=768, H=12 heads (head_dim 64), fp32 in/out.
Sharding: batch x head-group. Core c handles batch c//4 and heads
3*(c%4) .. 3*(c%4)+2 (Megatron column-parallel QKV, row-parallel Wo).
Each core computes a partial [2048, 768] output; the host sums the 4
partials per batch and adds bo.

All on-chip compute is bf16 (matmul accumulation fp32 in PSUM), which
keeps end-to-end absmax-rel error ~1e-3 against the fp32 reference.
The host pre-transposes x to x^T and pre-casts all weights to bf16, so
the device does no transposes at all.

Per-core steady state is Activation-engine bound: softmax needs
exp() of 3 heads x 2048^2 = 12.6M scores, and ScalarE runs 1 elem/
cycle/lane at 1.2 GHz regardless of dtype (~110us in [128,1024]
chunks).  Everything else (PE ~100us, DVE ~60us, DMA ~35us) is
overlapped under it:
  - JIT projections: only Q(qh0)+K(k0..7) run before the first score
    matmul, so the first exp issues ~9us in.
  - scores per k-tile: A on PE rows 0-63, B on rows 64-127, C kept at
    rows 64-127 (its K/Q live on partitions 64-127) so consecutive
    matmuls pair on disjoint row groups.
  - attn@V accumulates 4 k-tiles in PSUM ([65,512] chunks, V carrying
    a ones-column for the softmax denominators), then one DVE add into
    an SBUF accumulator.
  - Wo for query-half 0 is woven into query-half 1's k-loop.
"""

import numpy as np

_CACHE = {}

S = 2048
D = 768
HLOC = 3          # heads per core
NKT = 6           # 768 / 128 d-tiles
QH = 1024         # query half width


def _emit(nc, tc, ctx, dram, loop_n=None, phase=4):
    import concourse.mybir as mybir

    f32 = mybir.dt.float32
    bf16 = mybir.dt.bfloat16
    add = mybir.AluOpType.add
    mult = mybir.AluOpType.mult
    Exp = mybir.ActivationFunctionType.Exp

    consts = ctx.enter_context(tc.tile_pool(name="consts", bufs=1))
    ppool = ctx.enter_context(tc.tile_pool(name="ppool", bufs=24))
    pspool = ctx.enter_context(tc.tile_pool(name="pspool", bufs=3, space="PSUM"))
    pwork = ctx.enter_context(tc.tile_pool(name="pwork", bufs=2, space="PSUM"))
    opool = ctx.enter_context(tc.tile_pool(name="opool", bufs=2))
    bpool = ctx.enter_context(tc.tile_pool(name="bpool", bufs=2))
    rpool = ctx.enter_context(tc.tile_pool(name="rpool", bufs=2))

    # ---- persistent SBUF tensors ----
    xt = consts.tile([128, NKT, S], bf16)        # x^T (DMA'd pre-transposed)
    qt = consts.tile([128, S], bf16)             # Q^T heads A,B (scaled 1/8)
    kt = consts.tile([128, S], bf16)             # K^T heads A,B
    qtc = consts.tile([128, S], bf16)            # Q^T head C (rows 64:128)
    ktc = consts.tile([128, S], bf16)            # K^T head C (rows 64:128)
    v_sb = consts.tile([128, 16, HLOC, 72], bf16)  # V natural + ones col at 64
    acc = consts.tile([65, HLOC, S], f32)        # attn@V accumulator + denom
    attnT = consts.tile([128, S], bf16)          # normalized attn out^T A,B
    attnTc = consts.tile([64, S], bf16)          # head C

    w_qsb = consts.tile([128, NKT, 128], bf16)
    w_ksb = consts.tile([128, NKT, 128], bf16)
    w_qkc = consts.tile([128, NKT, 128], bf16)   # [Wq_C | Wk_C]
    w_vsb = consts.tile([128, NKT, 192], bf16)
    w_oab = consts.tile([128, D], bf16)
    w_oc = consts.tile([64, D], bf16)
    bq1 = consts.tile([128, 1], f32)
    bq2 = consts.tile([64, 1], f32)
    bk1 = consts.tile([128, 1], f32)
    bkc = consts.tile([128, 1], f32)             # rows 64:128 hold bk_C
    bv_bc = consts.tile([128, HLOC * 64], f32)
    dmy = consts.tile([1, 8], f32)
    dmy2 = consts.tile([1, 8], f32)

    # ---- prologue: warm the exp table while weights stream in ----
    nc.vector.memset(dmy, 0.0)
    nc.scalar.activation(out=dmy2, in_=dmy, func=Exp)

    # weight/bias DMAs ride the Activation queue (idle until first exp)
    nc.scalar.dma_start(out=w_qsb, in_=dram["w_qsb"])
    nc.scalar.dma_start(out=w_ksb, in_=dram["w_ksb"])
    nc.scalar.dma_start(out=w_qkc, in_=dram["w_qkc"])
    nc.scalar.dma_start(out=w_vsb, in_=dram["w_vsb"])
    nc.scalar.dma_start(out=w_oab, in_=dram["wo_ab"])
    nc.scalar.dma_start(out=w_oc, in_=dram["wo_c"])
    nc.scalar.dma_start(out=bq1, in_=dram["bq1"])
    nc.scalar.dma_start(out=bq2, in_=dram["bq2"])
    nc.scalar.dma_start(out=bk1, in_=dram["bk1"])
    nc.scalar.dma_start(out=bkc[64:128, :], in_=dram["bkc"])
    nc.scalar.dma_start(out=bv_bc, in_=dram["bv_bc"])

    ones_bf = consts.tile([128, 16 * HLOC], bf16)
    nc.vector.memset(ones_bf, 1.0)
    nc.vector.tensor_copy(
        out=v_sb[:, :, :, 64:65],
        in_=ones_bf.rearrange("p (a b c) -> p a b c", b=HLOC, c=1))

    xd = dram["xt"]

    def body():
        # ---- input DMAs (sync queue) ----
        for h in range(2):
            for c in range(2):
                cs = slice(h * 1024 + c * 512, h * 1024 + (c + 1) * 512)
                for dt in range(NKT):
                    nc.sync.dma_start(out=xt[:, dt, cs], in_=xd[:, dt, cs])

        # ---- JIT projection helpers ----
        nm = iter(range(10000))

        def proj_qk(wsb, cols, dst, bias, scale=None):
            pp = pwork.tile([128, 512], f32, name=f"pp_{next(nm)}", tag="w")
            for dt in range(NKT):
                nc.tensor.matmul(pp, lhsT=wsb[:, dt, :], rhs=xt[:, dt, cols],
                                 start=(dt == 0), stop=(dt == NKT - 1))
            if scale is None:
                nc.vector.tensor_scalar_add(dst, pp, bias)
            else:
                nc.vector.tensor_scalar(dst, pp, bias, scale, add, mult)

        def proj_c(c2):
            # merged [Q_C | K_C] for 512 source positions
            cols = slice(c2 * 512, (c2 + 1) * 512)
            pp = pwork.tile([128, 512], f32, name=f"ppc_{c2}", tag="w")
            for dt in range(NKT):
                nc.tensor.matmul(pp, lhsT=w_qkc[:, dt, :], rhs=xt[:, dt, cols],
                                 start=(dt == 0), stop=(dt == NKT - 1))
            nc.vector.tensor_scalar(qtc[0:64, cols], pp[0:64, :], bq2, 0.125,
                                    add, mult)
            nc.vector.tensor_scalar_add(ktc[64:128, cols], pp[64:128, :],
                                        bkc[64:128, :])
            # move Q_C up to rows 64:128 to pair C's score matmuls there
            nc.vector.tensor_copy(out=qtc[64:128, cols], in_=qtc[0:64, cols])

        def proj_v(sti):
            cols = slice(sti * 128, (sti + 1) * 128)
            pv = pwork.tile([128, 192], f32, name=f"pv_{sti}", tag="w")
            for dt in range(NKT):
                nc.tensor.matmul(pv, lhsT=xt[:, dt, cols], rhs=w_vsb[:, dt, :],
                                 start=(dt == 0), stop=(dt == NKT - 1))
            nc.vector.tensor_tensor(
                out=v_sb[:, sti, :, 0:64],
                in0=pv.rearrange("p (h d) -> p h d", h=HLOC),
                in1=bv_bc.rearrange("p (h d) -> p h d", h=HLOC),
                op=add)

        # ---- attention pieces ----
        p_tiles = {}

        def scores_exp(kti, qh):
            if phase < 2:
                return
            qs = slice(qh * QH, (qh + 1) * QH)
            ks = slice(kti * 128, (kti + 1) * 128)
            for h, (lh, rh, base) in enumerate(
                    ((kt, qt, 0), (kt, qt, 64), (ktc, qtc, 64))):
                ps = pspool.tile([128, QH], f32, name=f"ps{h}_{kti}_{qh}",
                                 tag="ps")
                for c in range(2):
                    nc.tensor.matmul(
                        ps[:, c * 512:(c + 1) * 512],
                        lhsT=lh[base:base + 64, ks],
                        rhs=rh[base:base + 64,
                               qh * QH + c * 512: qh * QH + (c + 1) * 512],
                        start=True, stop=True)
                p_t = ppool.tile([128, QH], bf16, name=f"p{h}_{kti}_{qh}",
                                 tag="p")
                nc.scalar.activation(out=p_t, in_=ps, func=Exp)
                p_tiles[(h, kti)] = p_t

        def attn_span(kg, qh, c, h):
            """One attn@V accumulation: head h, 512-query chunk c, k-group kg."""
            if phase < 3:
                return
            po = pwork.tile([65, 512], f32, name=f"po_{h}_{c}_{kg}_{qh}",
                            tag="w")
            for i, kti in enumerate(range(4 * kg, 4 * kg + 4)):
                nc.tensor.matmul(
                    po, lhsT=v_sb[:, kti, h, 0:65],
                    rhs=p_tiles[(h, kti)][:, c * 512:(c + 1) * 512],
                    start=(i == 0), stop=(i == 3))
            dst = acc[:, h, qh * QH + c * 512: qh * QH + (c + 1) * 512]
            if kg == 0:
                nc.vector.tensor_copy(out=dst, in_=po)
            else:
                nc.vector.tensor_tensor(out=dst, in0=dst, in1=po, op=add)

        def norm_chunk(qh, c):
            if phase < 3:
                return
            qs = slice(qh * QH + c * 512, qh * QH + (c + 1) * 512)
            for h in range(HLOC):
                r_t = rpool.tile([1, 512], f32, name=f"r_{h}_{qh}_{c}", tag="r")
                nc.vector.reciprocal(out=r_t, in_=acc[64:65, h, qs])
                b_t = bpool.tile([64, 512], f32, name=f"b_{h}_{qh}_{c}", tag="b")
                nc.gpsimd.partition_broadcast(b_t, r_t)
                if h == 0:
                    dst = attnT[0:64, qs]
                elif h == 1:
                    dst = attnT[64:128, qs]
                else:
                    dst = attnTc[0:64, qs]
                nc.vector.tensor_tensor(out=dst, in0=acc[0:64, h, qs], in1=b_t,
                                        op=mult)

        def emit_wo(stis):
            if phase < 4:
                return
            for sti in stis:
                ssl = slice(sti * 128, (sti + 1) * 128)
                o_t = opool.tile([128, D], f32, name=f"o_{sti}", tag="o")
                for e in range(2):
                    esl = slice(e * 384, (e + 1) * 384)
                    pw = pwork.tile([128, 384], f32, name=f"pwo_{sti}_{e}",
                                    tag="w")
                    nc.tensor.matmul(pw, lhsT=attnT[:, ssl], rhs=w_oab[:, esl],
                                     start=True, stop=False)
                    nc.tensor.matmul(pw, lhsT=attnTc[0:64, ssl],
                                     rhs=w_oc[0:64, esl],
                                     start=False, stop=True)
                    nc.vector.tensor_copy(out=o_t[:, esl], in_=pw)
                nc.sync.dma_start(out=dram["out"][ssl, :], in_=o_t)

        # ---- emission order = pipeline order ----
        # minimal prologue: first score matmul (k0, qh0) needs Q(qh0),
        # Q_C(qh0), and K cols 0:512 only
        proj_qk(w_qsb, slice(0, 512), qt[:, 0:512], bq1, 0.125)
        proj_qk(w_qsb, slice(512, 1024), qt[:, 512:1024], bq1, 0.125)
        proj_c(0)
        proj_c(1)
        proj_qk(w_ksb, slice(0, 512), kt[:, 0:512], bk1)

        # Flat k-tile loop over both query halves.  Window w covers the 4
        # k-tiles of group (w % 4) for query half (w // 4); between each
        # k-tile's scores+exp we drain a few deferred items — the previous
        # k-group's attn@V spans and JIT projections the NEXT window needs —
        # so the Activation queue never sits behind a long PE backlog.
        def spans(kg, qh):
            out = []
            for c in range(2):
                for h in range(HLOC):
                    out.append(lambda kg=kg, qh=qh, c=c, h=h:
                               attn_span(kg, qh, c, h))
            return out

        jits = {
            0: [lambda: proj_qk(w_ksb, slice(512, 1024), kt[:, 512:1024], bk1)]
               + [lambda s=s: proj_v(s) for s in range(0, 4)],
            1: [lambda: proj_qk(w_ksb, slice(1024, 1536), kt[:, 1024:1536],
                                bk1),
                lambda: proj_c(2)]
               + [lambda s=s: proj_v(s) for s in range(4, 8)],
            2: [lambda: proj_qk(w_ksb, slice(1536, 2048), kt[:, 1536:2048],
                                bk1),
                lambda: proj_c(3)]
               + [lambda s=s: proj_v(s) for s in range(8, 12)],
            3: [lambda: proj_qk(w_qsb, slice(1024, 1536), qt[:, 1024:1536],
                                bq1, 0.125),
                lambda: proj_qk(w_qsb, slice(1536, 2048), qt[:, 1536:2048],
                                bq1, 0.125)]
               + [lambda s=s: proj_v(s) for s in range(12, 16)],
            4: [lambda: norm_chunk(0, 0), lambda: norm_chunk(0, 1)],
            5: [lambda s=s: emit_wo([s]) for s in range(0, 4)],
            6: [lambda s=s: emit_wo([s]) for s in range(4, 8)],
            7: [],
        }

        for w in range(8):
            qh, kg = divmod(w, 4)
            # deferred: previous window's attn@V spans first (their p tiles
            # must be freed before this window's exps recycle the slots),
            # then this window's JIT projections.
            if w == 0:
                items = list(jits[0])
            elif w == 4:
                # qh boundary: spans of (kg3, qh0), then norm(0) chunks
                items = spans(3, 0) + jits[4]
            else:
                items = spans((w - 1) % 4, (w - 1) // 4) + jits[w]
            for i, kti in enumerate(range(4 * kg, 4 * kg + 4)):
                if w == 0:
                    scores_exp(kti, qh)
                for fn in items[i::4]:
                    fn()
                if w > 0:
                    scores_exp(kti, qh)

        # tail: last k-group of qh1, chunk-pipelined into norm + Wo
        for h in range(HLOC):
            attn_span(3, 1, 0, h)
        norm_chunk(1, 0)
        for h in range(HLOC):
            attn_span(3, 1, 1, h)
        emit_wo(range(8, 12))
        norm_chunk(1, 1)
        emit_wo(range(12, 16))

    if loop_n is None:
        body()
    else:
        with tc.For_i(0, loop_n, 1):
            body()


def _build(loop_n=None, phase=4):
    from contextlib import ExitStack

    import concourse.bacc as bacc
    import concourse.mybir as mybir
    import concourse.tile as tile

    f32 = mybir.dt.float32
    bf16 = mybir.dt.bfloat16
    nc = bacc.Bacc("TRN2", target_bir_lowering=False, debug=False, num_devices=8)
    dram = {
        "xt": nc.dram_tensor("xt", [128, NKT, S], bf16, kind="ExternalInput").ap(),
        "w_qsb": nc.dram_tensor("w_qsb", [128, NKT, 128], bf16,
                                kind="ExternalInput").ap(),
        "w_ksb": nc.dram_tensor("w_ksb", [128, NKT, 128], bf16,
                                kind="ExternalInput").ap(),
        "w_qkc": nc.dram_tensor("w_qkc", [128, NKT, 128], bf16,
                                kind="ExternalInput").ap(),
        "w_vsb": nc.dram_tensor("w_vsb", [128, NKT, 192], bf16,
                                kind="ExternalInput").ap(),
        "wo_ab": nc.dram_tensor("wo_ab", [128, D], bf16,
                                kind="ExternalInput").ap(),
        "wo_c": nc.dram_tensor("wo_c", [64, D], bf16, kind="ExternalInput").ap(),
        "bq1": nc.dram_tensor("bq1", [128, 1], f32, kind="ExternalInput").ap(),
        "bq2": nc.dram_tensor("bq2", [64, 1], f32, kind="ExternalInput").ap(),
        "bk1": nc.dram_tensor("bk1", [128, 1], f32, kind="ExternalInput").ap(),
        "bkc": nc.dram_tensor("bkc", [64, 1], f32, kind="ExternalInput").ap(),
        "bv_bc": nc.dram_tensor("bv_bc", [128, 192], f32,
                                kind="ExternalInput").ap(),
        "out": nc.dram_tensor("out", [S, D], f32, kind="ExternalOutput").ap(),
    }
    with tile.TileContext(nc) as tc:
        with ExitStack() as ctx:
            _emit(nc, tc, ctx, dram, loop_n=loop_n, phase=phase)
    nc.compile()
    return nc


def _get_nc():
    if "nc" not in _CACHE:
        _CACHE["nc"] = _build()
    return _CACHE["nc"]


def _shard(inputs):
    import ml_dtypes

    bf = ml_dtypes.bfloat16
    x = np.asarray(inputs["x"], np.float32)
    Wq = np.asarray(inputs["Wq"], np.float32)
    Wk = np.asarray(inputs["Wk"], np.float32)
    Wv = np.asarray(inputs["Wv"], np.float32)
    Wo = np.asarray(inputs["Wo"], np.float32)
    bq = np.asarray(inputs["bq"], np.float32)
    bk = np.asarray(inputs["bk"], np.float32)
    bv = np.asarray(inputs["bv"], np.float32)

    def wtiles(w):  # [768, C] -> [128, 6, C]
        return np.ascontiguousarray(
            w.reshape(NKT, 128, -1).transpose(1, 0, 2)).astype(bf)

    xts = []
    for b in range(2):
        xts.append(np.ascontiguousarray(
            x[b].T.reshape(NKT, 128, S).transpose(1, 0, 2)).astype(bf))

    in_maps = []
    for c in range(8):
        b, g = divmod(c, 4)
        o = 192 * g
        in_maps.append({
            "xt": xts[b],
            "w_qsb": wtiles(Wq[:, o:o + 128]),
            "w_ksb": wtiles(Wk[:, o:o + 128]),
            "w_qkc": wtiles(np.concatenate(
                [Wq[:, o + 128:o + 192], Wk[:, o + 128:o + 192]], axis=1)),
            "w_vsb": wtiles(Wv[:, o:o + 192]),
            "wo_ab": np.ascontiguousarray(Wo[o:o + 128, :]).astype(bf),
            "wo_c": np.ascontiguousarray(Wo[o + 128:o + 192, :]).astype(bf),
            "bq1": np.ascontiguousarray(bq[o:o + 128, None]),
            "bq2": np.ascontiguousarray(bq[o + 128:o + 192, None]),
            "bk1": np.ascontiguousarray(bk[o:o + 128, None]),
            "bkc": np.ascontiguousarray(bk[o + 128:o + 192, None]),
            "bv_bc": np.ascontiguousarray(
                np.broadcast_to(bv[o:o + 192], (128, 192))),
        })
    return in_maps


def kernel(x, Wq, bq, Wk, bk, Wv, bv, Wo, bo):
    from concourse.bass_utils import run_bass_kernel_spmd

    nc = _get_nc()
    in_maps = _shard(dict(x=x, Wq=Wq, Wk=Wk, Wv=Wv, Wo=Wo,
                          bq=bq, bk=bk, bv=bv))
    res = run_bass_kernel_spmd(nc, in_maps, core_ids=list(range(8)))
    out = np.zeros((2, S, D), np.float32)
    for c in range(8):
        out[c // 4] += res.results[c]["out"]
    out += np.asarray(bo, np.float32)
    return out


# revision 16
# speedup vs baseline: 1.2909x; 1.0680x over previous
"""Multi-head self-attention (no causal mask) on 8 Trainium2 NeuronCores.

Problem: B=2, S=2048, D=768, H=12 heads (head_dim 64), fp32 in/out.
Sharding: batch x head-group. Core c handles batch c//4 and heads
3*(c%4) .. 3*(c%4)+2 (Megatron column-parallel QKV, row-parallel Wo).
Each core computes a partial [2048, 768] output; the host sums the 4
partials per batch and adds bo.

All on-chip compute is bf16 (matmul accumulation fp32 in PSUM), which
keeps end-to-end absmax-rel error ~1e-3 against the fp32 reference.
The host pre-transposes x to x^T and pre-casts all weights to bf16, so
the device does no transposes at all.

Per-core steady state is Activation-engine bound: softmax needs
exp() of 3 heads x 2048^2 = 12.6M scores, and ScalarE runs 1 elem/
cycle/lane at 1.2 GHz regardless of dtype (~110us in [128,1024]
chunks).  Everything else (PE ~100us, DVE ~60us, DMA ~35us) is
overlapped under it:
  - JIT projections: only Q(qh0)+K(k0..7) run before the first score
    matmul, so the first exp issues ~9us in.
  - scores per k-tile: A on PE rows 0-63, B on rows 64-127, C kept at
    rows 64-127 (its K/Q live on partitions 64-127) so consecutive
    matmuls pair on disjoint row groups.
  - attn@V accumulates 4 k-tiles in PSUM ([65,512] chunks, V carrying
    a ones-column for the softmax denominators), then one DVE add into
    an SBUF accumulator.
  - Wo for query-half 0 is woven into query-half 1's k-loop.
"""

import numpy as np

_CACHE = {}

S = 2048
D = 768
HLOC = 3          # heads per core
NKT = 6           # 768 / 128 d-tiles
QH = 1024         # query half width


def _emit(nc, tc, ctx, dram, loop_n=None, phase=4):
    import concourse.mybir as mybir

    f32 = mybir.dt.float32
    bf16 = mybir.dt.bfloat16
    add = mybir.AluOpType.add
    mult = mybir.AluOpType.mult
    Exp = mybir.ActivationFunctionType.Exp

    consts = ctx.enter_context(tc.tile_pool(name="consts", bufs=1))
    ppool = ctx.enter_context(tc.tile_pool(name="ppool", bufs=32))
    pspool = ctx.enter_context(tc.tile_pool(name="pspool", bufs=3, space="PSUM"))
    pwork = ctx.enter_context(tc.tile_pool(name="pwork", bufs=2, space="PSUM"))
    opool = ctx.enter_context(tc.tile_pool(name="opool", bufs=2))
    bpool = ctx.enter_context(tc.tile_pool(name="bpool", bufs=2))
    rpool = ctx.enter_context(tc.tile_pool(name="rpool", bufs=2))

    # ---- persistent SBUF tensors ----
    xt = consts.tile([128, NKT, S], bf16)        # x^T (DMA'd pre-transposed)
    qt = consts.tile([128, S], bf16)             # Q^T heads A,B (scaled 1/8)
    kt = consts.tile([128, S], bf16)             # K^T heads A,B
    qtc = consts.tile([128, S], bf16)            # Q^T head C (rows 64:128)
    ktc = consts.tile([128, S], bf16)            # K^T head C (rows 64:128)
    v_sb = consts.tile([128, 16, HLOC, 72], bf16)  # V natural + ones col at 64
    acc = consts.tile([65, HLOC, S], f32)        # attn@V accumulator + denom
    attnT = consts.tile([128, S], bf16)          # normalized attn out^T A,B
    attnTc = consts.tile([64, S], bf16)          # head C

    w_qsb = consts.tile([128, NKT, 128], bf16)
    w_ksb = consts.tile([128, NKT, 128], bf16)
    w_qkc = consts.tile([128, NKT, 128], bf16)   # [Wq_C | Wk_C]
    w_vsb = consts.tile([128, NKT, 192], bf16)
    w_oab = consts.tile([128, D], bf16)
    w_oc = consts.tile([64, D], bf16)
    bq1 = consts.tile([128, 1], f32)
    bq2 = consts.tile([64, 1], f32)
    bk1 = consts.tile([128, 1], f32)
    bkc = consts.tile([128, 1], f32)             # rows 64:128 hold bk_C
    bv_bc = consts.tile([128, HLOC * 64], f32)
    dmy = consts.tile([1, 8], f32)
    dmy2 = consts.tile([1, 8], f32)

    # ---- prologue: warm the exp table while weights stream in ----
    nc.vector.memset(dmy, 0.0)
    nc.scalar.activation(out=dmy2, in_=dmy, func=Exp)

    # weight/bias DMAs ride the Activation queue (idle until first exp)
    nc.scalar.dma_start(out=w_qsb, in_=dram["w_qsb"])
    nc.scalar.dma_start(out=w_ksb, in_=dram["w_ksb"])
    nc.scalar.dma_start(out=w_qkc, in_=dram["w_qkc"])
    nc.scalar.dma_start(out=w_vsb, in_=dram["w_vsb"])
    nc.scalar.dma_start(out=w_oab, in_=dram["wo_ab"])
    nc.scalar.dma_start(out=w_oc, in_=dram["wo_c"])
    nc.scalar.dma_start(out=bq1, in_=dram["bq1"])
    nc.scalar.dma_start(out=bq2, in_=dram["bq2"])
    nc.scalar.dma_start(out=bk1, in_=dram["bk1"])
    nc.scalar.dma_start(out=bkc[64:128, :], in_=dram["bkc"])
    nc.scalar.dma_start(out=bv_bc, in_=dram["bv_bc"])

    ones_bf = consts.tile([128, 16 * HLOC], bf16)
    nc.vector.memset(ones_bf, 1.0)
    nc.vector.tensor_copy(
        out=v_sb[:, :, :, 64:65],
        in_=ones_bf.rearrange("p (a b c) -> p a b c", b=HLOC, c=1))

    xd = dram["xt"]

    def body():
        # ---- input DMAs (sync queue) ----
        for h in range(2):
            for c in range(2):
                cs = slice(h * 1024 + c * 512, h * 1024 + (c + 1) * 512)
                for dt in range(NKT):
                    nc.sync.dma_start(out=xt[:, dt, cs], in_=xd[:, dt, cs])

        # ---- JIT projection helpers ----
        nm = iter(range(10000))

        def proj_qk(wsb, cols, dst, bias, scale=None):
            pp = pwork.tile([128, 512], f32, name=f"pp_{next(nm)}", tag="w")
            for dt in range(NKT):
                nc.tensor.matmul(pp, lhsT=wsb[:, dt, :], rhs=xt[:, dt, cols],
                                 start=(dt == 0), stop=(dt == NKT - 1))
            if scale is None:
                nc.vector.tensor_scalar_add(dst, pp, bias)
            else:
                nc.vector.tensor_scalar(dst, pp, bias, scale, add, mult)

        def proj_c(c2):
            # merged [Q_C | K_C] for 512 source positions
            cols = slice(c2 * 512, (c2 + 1) * 512)
            pp = pwork.tile([128, 512], f32, name=f"ppc_{c2}", tag="w")
            for dt in range(NKT):
                nc.tensor.matmul(pp, lhsT=w_qkc[:, dt, :], rhs=xt[:, dt, cols],
                                 start=(dt == 0), stop=(dt == NKT - 1))
            nc.vector.tensor_scalar(qtc[0:64, cols], pp[0:64, :], bq2, 0.125,
                                    add, mult)
            nc.vector.tensor_scalar_add(ktc[64:128, cols], pp[64:128, :],
                                        bkc[64:128, :])
            # move Q_C up to rows 64:128 to pair C's score matmuls there
            nc.vector.tensor_copy(out=qtc[64:128, cols], in_=qtc[0:64, cols])

        def proj_v(sti):
            cols = slice(sti * 128, (sti + 1) * 128)
            pv = pwork.tile([128, 192], f32, name=f"pv_{sti}", tag="w")
            for dt in range(NKT):
                nc.tensor.matmul(pv, lhsT=xt[:, dt, cols], rhs=w_vsb[:, dt, :],
                                 start=(dt == 0), stop=(dt == NKT - 1))
            nc.vector.tensor_tensor(
                out=v_sb[:, sti, :, 0:64],
                in0=pv.rearrange("p (h d) -> p h d", h=HLOC),
                in1=bv_bc.rearrange("p (h d) -> p h d", h=HLOC),
                op=add)

        # ---- attention pieces ----
        p_tiles = {}

        def scores_exp(kti, qh):
            if phase < 2:
                return
            qs = slice(qh * QH, (qh + 1) * QH)
            ks = slice(kti * 128, (kti + 1) * 128)
            for h, (lh, rh, base) in enumerate(
                    ((kt, qt, 0), (kt, qt, 64), (ktc, qtc, 64))):
                ps = pspool.tile([128, QH], f32, name=f"ps{h}_{kti}_{qh}",
                                 tag="ps")
                for c in range(2):
                    nc.tensor.matmul(
                        ps[:, c * 512:(c + 1) * 512],
                        lhsT=lh[base:base + 64, ks],
                        rhs=rh[base:base + 64,
                               qh * QH + c * 512: qh * QH + (c + 1) * 512],
                        start=True, stop=True)
                p_t = ppool.tile([128, QH], bf16, name=f"p{h}_{kti}_{qh}",
                                 tag="p")
                nc.scalar.activation(out=p_t, in_=ps, func=Exp)
                p_tiles[(h, kti)] = p_t

        def attn_span(kg, qh, c, h):
            """One attn@V accumulation: head h, 512-query chunk c, k-group kg."""
            if phase < 3:
                return
            po = pwork.tile([65, 512], f32, name=f"po_{h}_{c}_{kg}_{qh}",
                            tag="w")
            for i, kti in enumerate(range(4 * kg, 4 * kg + 4)):
                nc.tensor.matmul(
                    po, lhsT=v_sb[:, kti, h, 0:65],
                    rhs=p_tiles[(h, kti)][:, c * 512:(c + 1) * 512],
                    start=(i == 0), stop=(i == 3))
            dst = acc[:, h, qh * QH + c * 512: qh * QH + (c + 1) * 512]
            if kg == 0:
                nc.vector.tensor_copy(out=dst, in_=po)
            else:
                nc.vector.tensor_tensor(out=dst, in0=dst, in1=po, op=add)

        def norm_chunk(qh, c):
            if phase < 3:
                return
            qs = slice(qh * QH + c * 512, qh * QH + (c + 1) * 512)
            for h in range(HLOC):
                r_t = rpool.tile([1, 512], f32, name=f"r_{h}_{qh}_{c}", tag="r")
                nc.vector.reciprocal(out=r_t, in_=acc[64:65, h, qs])
                b_t = bpool.tile([64, 512], f32, name=f"b_{h}_{qh}_{c}", tag="b")
                nc.gpsimd.partition_broadcast(b_t, r_t)
                if h == 0:
                    dst = attnT[0:64, qs]
                elif h == 1:
                    dst = attnT[64:128, qs]
                else:
                    dst = attnTc[0:64, qs]
                nc.vector.tensor_tensor(out=dst, in0=acc[0:64, h, qs], in1=b_t,
                                        op=mult)

        def emit_wo(stis, eng=None):
            if phase < 4:
                return
            for sti in stis:
                ssl = slice(sti * 128, (sti + 1) * 128)
                o_t = opool.tile([128, D], f32, name=f"o_{sti}", tag="o")
                for e in range(2):
                    esl = slice(e * 384, (e + 1) * 384)
                    pw = pwork.tile([128, 384], f32, name=f"pwo_{sti}_{e}",
                                    tag="w")
                    nc.tensor.matmul(pw, lhsT=attnT[:, ssl], rhs=w_oab[:, esl],
                                     start=True, stop=False)
                    nc.tensor.matmul(pw, lhsT=attnTc[0:64, ssl],
                                     rhs=w_oc[0:64, esl],
                                     start=False, stop=True)
                    if eng is None:
                        nc.vector.tensor_copy(out=o_t[:, esl], in_=pw)
                    else:
                        eng.copy(out=o_t[:, esl], in_=pw)
                nc.sync.dma_start(out=dram["out"][ssl, :], in_=o_t)

        # ---- emission order = pipeline order ----
        # minimal prologue: first score matmul (k0, qh0) needs Q(qh0),
        # Q_C(qh0), and K cols 0:512 only
        proj_qk(w_qsb, slice(0, 512), qt[:, 0:512], bq1, 0.125)
        proj_qk(w_qsb, slice(512, 1024), qt[:, 512:1024], bq1, 0.125)
        proj_c(0)
        proj_c(1)
        proj_qk(w_ksb, slice(0, 512), kt[:, 0:512], bk1)

        # Flat k-tile loop over both query halves.  Window w covers the 4
        # k-tiles of group (w % 4) for query half (w // 4); between each
        # k-tile's scores+exp we drain a few deferred items — the previous
        # k-group's attn@V spans and JIT projections the NEXT window needs —
        # so the Activation queue never sits behind a long PE backlog.
        def spans(kg, qh):
            out = []
            for c in range(2):
                for h in range(HLOC):
                    out.append(lambda kg=kg, qh=qh, c=c, h=h:
                               attn_span(kg, qh, c, h))
            return out

        jits = {
            0: [lambda: proj_qk(w_ksb, slice(512, 1024), kt[:, 512:1024], bk1)]
               + [lambda s=s: proj_v(s) for s in range(0, 4)],
            1: [lambda: proj_qk(w_ksb, slice(1024, 1536), kt[:, 1024:1536],
                                bk1),
                lambda: proj_c(2)]
               + [lambda s=s: proj_v(s) for s in range(4, 8)],
            2: [lambda: proj_qk(w_ksb, slice(1536, 2048), kt[:, 1536:2048],
                                bk1),
                lambda: proj_c(3)]
               + [lambda s=s: proj_v(s) for s in range(8, 12)],
            3: [lambda: proj_qk(w_qsb, slice(1024, 1536), qt[:, 1024:1536],
                                bq1, 0.125),
                lambda: proj_qk(w_qsb, slice(1536, 2048), qt[:, 1536:2048],
                                bq1, 0.125)]
               + [lambda s=s: proj_v(s) for s in range(12, 16)],
            4: [lambda: norm_chunk(0, 0), lambda: norm_chunk(0, 1)],
            5: [lambda s=s: emit_wo([s]) for s in range(0, 4)],
            6: [lambda s=s: emit_wo([s]) for s in range(4, 8)],
            7: [],
        }

        for w in range(8):
            qh, kg = divmod(w, 4)
            # deferred: previous window's attn@V spans first (their p tiles
            # must be freed before this window's exps recycle the slots),
            # then this window's JIT projections.
            if w == 0:
                items = list(jits[0])
            elif w == 4:
                # qh boundary: spans of (kg3, qh0), then norm(0) chunks
                items = spans(3, 0) + jits[4]
            else:
                items = spans((w - 1) % 4, (w - 1) // 4) + jits[w]
            for i, kti in enumerate(range(4 * kg, 4 * kg + 4)):
                if w == 0:
                    scores_exp(kti, qh)
                for fn in items[i::4]:
                    fn()
                if w > 0:
                    scores_exp(kti, qh)

        # tail: last k-group of qh1, chunk-pipelined into norm + Wo
        for h in range(HLOC):
            attn_span(3, 1, 0, h)
        norm_chunk(1, 0)
        for h in range(HLOC):
            attn_span(3, 1, 1, h)
        emit_wo(range(8, 12), eng=nc.scalar)
        norm_chunk(1, 1)
        emit_wo(range(12, 16), eng=nc.scalar)

    if loop_n is None:
        body()
    else:
        with tc.For_i(0, loop_n, 1):
            body()


def _build(loop_n=None, phase=4):
    from contextlib import ExitStack

    import concourse.bacc as bacc
    import concourse.mybir as mybir
    import concourse.tile as tile

    f32 = mybir.dt.float32
    bf16 = mybir.dt.bfloat16
    nc = bacc.Bacc("TRN2", target_bir_lowering=False, debug=False, num_devices=8)
    dram = {
        "xt": nc.dram_tensor("xt", [128, NKT, S], bf16, kind="ExternalInput").ap(),
        "w_qsb": nc.dram_tensor("w_qsb", [128, NKT, 128], bf16,
                                kind="ExternalInput").ap(),
        "w_ksb": nc.dram_tensor("w_ksb", [128, NKT, 128], bf16,
                                kind="ExternalInput").ap(),
        "w_qkc": nc.dram_tensor("w_qkc", [128, NKT, 128], bf16,
                                kind="ExternalInput").ap(),
        "w_vsb": nc.dram_tensor("w_vsb", [128, NKT, 192], bf16,
                                kind="ExternalInput").ap(),
        "wo_ab": nc.dram_tensor("wo_ab", [128, D], bf16,
                                kind="ExternalInput").ap(),
        "wo_c": nc.dram_tensor("wo_c", [64, D], bf16, kind="ExternalInput").ap(),
        "bq1": nc.dram_tensor("bq1", [128, 1], f32, kind="ExternalInput").ap(),
        "bq2": nc.dram_tensor("bq2", [64, 1], f32, kind="ExternalInput").ap(),
        "bk1": nc.dram_tensor("bk1", [128, 1], f32, kind="ExternalInput").ap(),
        "bkc": nc.dram_tensor("bkc", [64, 1], f32, kind="ExternalInput").ap(),
        "bv_bc": nc.dram_tensor("bv_bc", [128, 192], f32,
                                kind="ExternalInput").ap(),
        "out": nc.dram_tensor("out", [S, D], f32, kind="ExternalOutput").ap(),
    }
    with tile.TileContext(nc) as tc:
        with ExitStack() as ctx:
            _emit(nc, tc, ctx, dram, loop_n=loop_n, phase=phase)
    nc.compile()
    return nc


def _get_nc():
    if "nc" not in _CACHE:
        _CACHE["nc"] = _build()
    return _CACHE["nc"]


def _shard(inputs):
    import ml_dtypes

    bf = ml_dtypes.bfloat16
    x = np.asarray(inputs["x"], np.float32)
    Wq = np.asarray(inputs["Wq"], np.float32)
    Wk = np.asarray(inputs["Wk"], np.float32)
    Wv = np.asarray(inputs["Wv"], np.float32)
    Wo = np.asarray(inputs["Wo"], np.float32)
    bq = np.asarray(inputs["bq"], np.float32)
    bk = np.asarray(inputs["bk"], np.float32)
    bv = np.asarray(inputs["bv"], np.float32)

    def wtiles(w):  # [768, C] -> [128, 6, C]
        return np.ascontiguousarray(
            w.reshape(NKT, 128, -1).transpose(1, 0, 2)).astype(bf)

    xts = []
    for b in range(2):
        xts.append(np.ascontiguousarray(
            x[b].T.reshape(NKT, 128, S).transpose(1, 0, 2)).astype(bf))

    in_maps = []
    for c in range(8):
        b, g = divmod(c, 4)
        o = 192 * g
        in_maps.append({
            "xt": xts[b],
            "w_qsb": wtiles(Wq[:, o:o + 128]),
            "w_ksb": wtiles(Wk[:, o:o + 128]),
            "w_qkc": wtiles(np.concatenate(
                [Wq[:, o + 128:o + 192], Wk[:, o + 128:o + 192]], axis=1)),
            "w_vsb": wtiles(Wv[:, o:o + 192]),
            "wo_ab": np.ascontiguousarray(Wo[o:o + 128, :]).astype(bf),
            "wo_c": np.ascontiguousarray(Wo[o + 128:o + 192, :]).astype(bf),
            "bq1": np.ascontiguousarray(bq[o:o + 128, None]),
            "bq2": np.ascontiguousarray(bq[o + 128:o + 192, None]),
            "bk1": np.ascontiguousarray(bk[o:o + 128, None]),
            "bkc": np.ascontiguousarray(bk[o + 128:o + 192, None]),
            "bv_bc": np.ascontiguousarray(
                np.broadcast_to(bv[o:o + 192], (128, 192))),
        })
    return in_maps


def kernel(x, Wq, bq, Wk, bk, Wv, bv, Wo, bo):
    from concourse.bass_utils import run_bass_kernel_spmd

    nc = _get_nc()
    in_maps = _shard(dict(x=x, Wq=Wq, Wk=Wk, Wv=Wv, Wo=Wo,
                          bq=bq, bk=bk, bv=bv))
    res = run_bass_kernel_spmd(nc, in_maps, core_ids=list(range(8)))
    out = np.zeros((2, S, D), np.float32)
    for c in range(8):
        out[c // 4] += res.results[c]["out"]
    out += np.asarray(bo, np.float32)
    return out
